# revision 1
# baseline (speedup 1.0000x reference)
"""Fused transformer block (attention + SwiGLU MLP, RMS norms) on 8 TRN2 NeuronCores.

Sharding: tensor-parallel attention over heads (2 heads/core, w_qkv column-split,
w_o row-split, attn_bias head-split) + tensor-parallel MLP over the SwiGLU
intermediate dim (352/core, zero-padded to 384). Two AllReduces combine the
o_proj and down_proj partials; norms/residuals are computed redundantly on all
cores in a transposed [feature, seq] layout so every matmul contracts along
SBUF partitions.

Host-side prep: activations/weights are pre-transposed; attn_bias is shipped as
exp(bias^T) in bf16 so softmax becomes exp(q k^T/8) * expbias with the row sums
taken by an appended ones-column in the PV matmul (no max-subtraction needed at
these input scales).
"""

import sys

sys.path.insert(0, "/opt/trn_rl_repo")

import numpy as np
import ml_dtypes

import concourse.bass as bass
import concourse.mybir as mybir
import concourse.tile as tile
from concourse import bacc
from concourse.bass_utils import run_bass_kernel_spmd
from concourse.masks import make_identity

P = 128
S = 2048
HID = 1024
NH = 16
HD = 64
INTER = 2816
EPS = 1e-5
N_CORES = 8
HPC = NH // N_CORES          # heads per core = 2
IP = 384                     # padded per-core intermediate (352 -> 384)
QC = 512                     # attention q-chunk
NQC = S // QC                # 8
KB = S // P                  # 16 k-blocks
KT = HID // P                # 8 hid k-tiles
F32 = mybir.dt.float32
F32R = mybir.dt.float32r
BF16 = mybir.dt.bfloat16

_cache = {}


def _build():
    nc = bacc.Bacc("TRN2", target_bir_lowering=False, debug=False,
                   num_devices=N_CORES)
    xT = nc.dram_tensor("xT", [HID, S], F32, kind="ExternalInput").ap()
    wqkv = nc.dram_tensor("wqkv", [HID, 3 * P], F32, kind="ExternalInput").ap()
    wo = nc.dram_tensor("wo", [P, HID], F32, kind="ExternalInput").ap()
    cs2 = nc.dram_tensor("cs2", [P, S], F32, kind="ExternalInput").ap()
    sn2 = nc.dram_tensor("sn2", [P, S], F32, kind="ExternalInput").ap()
    r2t = nc.dram_tensor("r2t", [P, P], F32, kind="ExternalInput").ap()
    expb = nc.dram_tensor("expb", [HPC, S, S], BF16, kind="ExternalInput").ap()
    wgu = nc.dram_tensor("wgu", [HID, 2 * IP], BF16, kind="ExternalInput").ap()
    wdn = nc.dram_tensor("wdn", [IP, HID], BF16, kind="ExternalInput").ap()
    outT = nc.dram_tensor("outT", [HID, S], F32, kind="ExternalOutput").ap()

    with tile.TileContext(nc) as tc:
        _body(nc, tc, xT, wqkv, wo, cs2, sn2, r2t, expb, wgu, wdn, outT)
    nc.compile()
    return nc


def _body(nc, tc, xT, wqkv, wo, cs2, sn2, r2t, expb, wgu, wdn, outT):
    # ---- full-kernel resident tensors ----
    with tc.tile_pool(name="const", bufs=1) as const, \
         tc.tile_pool(name="dram1", bufs=1, space="DRAM") as dram1:
        xt = const.tile([P, KT, S], F32, tag="xt")      # x^T -> x1^T -> x2^T
        xtbc = [const.tile([P, KT, 512], BF16, tag=f"xtb{j}", name=f"xtb{j}")
                for j in range(4)]                      # bf16 copy for matmuls
        misc = const.tile([P, 2], F32, tag="misc")      # eps scratch
        onesb = const.tile([P, 1], BF16, tag="onesb")
        onesr = const.tile([1, P], F32, tag="onesr")
        idb = const.tile([P, P], BF16, tag="idb")
        eps_sb = misc[0:1, 0:1]
        nc.gpsimd.memset(eps_sb, EPS)
        nc.gpsimd.memset(onesb[:], 1.0)
        nc.gpsimd.memset(onesr[:], 1.0)
        make_identity(nc, idb[:])
        xTr = xT.rearrange("(t p) s -> p t s", p=P)
        for j in range(4):
            sl = slice(j * 512, (j + 1) * 512)
            nc.sync.dma_start(xt[:, :, sl], xTr[:, :, sl])
            for t in range(KT):
                k = (j * KT + t) % (2 if j == 0 else 3)
                if k == 0:
                    nc.scalar.copy(xtbc[j][:, t, :], xt[:, t, sl])
                elif k == 1:
                    nc.vector.tensor_copy(xtbc[j][:, t, :], xt[:, t, sl])
                else:
                    nc.gpsimd.tensor_copy(xtbc[j][:, t, :], xt[:, t, sl])

        o1c = [dram1.tile([HID, 512], BF16, tag=f"o1c{j}", name=f"o1c{j}")
               for j in range(4)]
        o1sc = [dram1.tile([HID, 512], BF16, tag=f"o1sc{j}", name=f"o1sc{j}",
                           addr_space="Shared") for j in range(4)]
        o2c = [dram1.tile([HID, 512], BF16, tag=f"o2c{j}", name=f"o2c{j}")
               for j in range(4)]
        o2sc = [dram1.tile([HID, 512], BF16, tag=f"o2sc{j}", name=f"o2sc{j}",
                           addr_space="Shared") for j in range(4)]

        # ============ phase 1+2: qkv projection, rope, attention ============
        with tc.tile_pool(name="att", bufs=1) as att, \
             tc.tile_pool(name="wk_att", bufs=2) as wk:
            qT = att.tile([P, S], BF16, tag="qT")
            kTt = att.tile([P, S], BF16, tag="kT")
            vaug = att.tile([P, KB, 2 * (HD + 1)], BF16, tag="vaug")
            pT = [att.tile([P, KB, QC], BF16, tag=f"pT{h}", name=f"pT{h}")
                  for h in range(HPC)]
            wo_sb = att.tile([P, HID], BF16, tag="wo")
            nc.gpsimd.memset(vaug[:, :, HD], 1.0)
            nc.gpsimd.memset(vaug[:, :, 2 * HD + 1], 1.0)

            with tc.tile_pool(name="ph1", bufs=1) as ph1, \
                 tc.tile_pool(name="res_q", bufs=2) as resq, \
                 tc.tile_pool(name="ps_q", bufs=3, space="PSUM") as psq:
                wqf = ph1.tile([P, KT, 3 * P], F32, tag="wqf")
                nc.sync.dma_start(wqf[:], wqkv.rearrange("(t p) m -> p t m", p=P))
                wqb = ph1.tile([P, KT, 3 * P], BF16, tag="wqb")
                nc.vector.tensor_copy(wqb[:], wqf[:])
                nc.vector.tensor_copy(wo_sb[:], _dma_bf(nc, resq, wo, [P, HID], "wof"))
                vT_bf = ph1.tile([P, S], BF16, tag="vT")
                for n in range(4):                       # seq chunks of 512
                    sl = slice(n * 512, (n + 1) * 512)
                    for part in range(3):                # q, k, v column blocks
                        ps = psq.tile([P, 512], F32, tag="mm")
                        for kt in range(KT):
                            nc.tensor.matmul(
                                ps[:],
                                lhsT=wqb[:, kt, part * P:(part + 1) * P],
                                rhs=xtbc[n][:, kt, :],
                                start=(kt == 0), stop=(kt == KT - 1),
                            )
                        if part == 0:
                            nc.scalar.mul(qT[:, sl], ps[:], 0.125)  # fold 1/sqrt(HD)
                        elif part == 1:
                            nc.scalar.copy(kTt[:, sl], ps[:])
                        else:
                            nc.scalar.copy(vT_bf[:, sl], ps[:])

                # ---- RoPE on q and k:  t <- t*cos + (R2 @ t)*sin ----
                cs_sb = ph1.tile([P, S], BF16, tag="cs")
                sn_sb = ph1.tile([P, S], BF16, tag="sn")
                nc.vector.tensor_copy(cs_sb[:], _dma_bf(nc, resq, cs2, [P, S], "csf"))
                nc.vector.tensor_copy(sn_sb[:], _dma_bf(nc, resq, sn2, [P, S], "snf"))
                r2b = ph1.tile([P, P], BF16, tag="r2b")
                nc.vector.tensor_copy(r2b[:], _dma_bf(nc, resq, r2t, [P, P], "r2f"))
                for t_sb in (qT, kTt):
                    for n in range(4):
                        sl = slice(n * 512, (n + 1) * 512)
                        psr = psq.tile([P, 512], F32, tag="mm")
                        nc.tensor.matmul(psr[:], lhsT=r2b[:],
                                         rhs=t_sb[:, sl], start=True, stop=True)
                        m1 = resq.tile([P, 512], BF16, tag="w512")
                        m2 = resq.tile([P, 512], BF16, tag="w512b")
                        nc.vector.tensor_mul(out=m1[:], in0=t_sb[:, sl], in1=cs_sb[:, sl])
                        nc.vector.tensor_mul(out=m2[:], in0=psr[:], in1=sn_sb[:, sl])
                        nc.vector.tensor_add(out=t_sb[:, sl], in0=m1[:], in1=m2[:])

                # ---- v_aug: transpose v^T into [k, (v_h | 1)] blocks ----
                with tc.tile_pool(name="ps_t", bufs=2, space="PSUM") as pst:
                    for kb in range(KB):
                        pt = pst.tile([P, P], BF16, tag="tr")
                        nc.tensor.transpose(pt[:], vT_bf[:, kb * P:(kb + 1) * P], idb[:])
                        nc.vector.tensor_copy(vaug[:, kb, 0:HD], pt[:, 0:HD])
                        nc.vector.tensor_copy(vaug[:, kb, HD + 1:2 * HD + 1],
                                              pt[:, HD:2 * HD])

            # ---- attention proper ----
            with tc.tile_pool(name="ps_a", bufs=3, space="PSUM") as psa, \
                 tc.tile_pool(name="ps_pv", bufs=2, space="PSUM") as pspv, \
                 tc.tile_pool(name="ps_o", bufs=2, space="PSUM") as pso, \
                 tc.tile_pool(name="ps_zb", bufs=1, space="PSUM") as pszb, \
                 tc.tile_pool(name="eb_p", bufs=6) as ebp:
                for n in range(NQC):
                    qsl = slice(n * QC, (n + 1) * QC)
                    for kb in range(KB):
                        for h in range(HPC):
                            hsl = slice(h * HD, (h + 1) * HD)
                            ps = psa.tile([P, QC], F32, tag="qk")
                            nc.tensor.matmul(
                                ps[:],
                                lhsT=kTt[hsl, kb * P:(kb + 1) * P],
                                rhs=qT[hsl, qsl],
                                start=True, stop=True,
                                tile_position=(h * HD, 0),
                            )
                            eb = ebp.tile([P, QC], BF16, tag="eb")
                            nc.sync.dma_start(eb[:], expb[h, kb * P:(kb + 1) * P, qsl])
                            nc.scalar.activation(pT[h][:, kb, :], ps[:],
                                                 mybir.ActivationFunctionType.Exp)
                            nc.vector.tensor_mul(out=pT[h][:, kb, :],
                                                 in0=pT[h][:, kb, :], in1=eb[:])
                    aoT = wk.tile([P, QC], BF16, tag="ao")
                    for h in range(HPC):
                        pv = pspv.tile([HD + 1, QC], F32, tag="pv")
                        a0 = h * (HD + 1)
                        for kb in range(KB):
                            nc.tensor.matmul(
                                pv[:],
                                lhsT=vaug[:, kb, a0:a0 + HD + 1],
                                rhs=pT[h][:, kb, :],
                                start=(kb == 0), stop=(kb == KB - 1),
                            )
                        zrow = wk.tile([1, QC], F32, tag="zrow")
                        nc.vector.reciprocal(zrow[:], pv[HD:HD + 1, :])
                        pzb = pszb.tile([HD, QC], F32, tag="zbp")
                        nc.tensor.matmul(pzb[:], lhsT=onesr[:, :HD], rhs=zrow[:],
                                         start=True, stop=True)
                        zb = wk.tile([HD, QC], F32, tag="zb")
                        nc.scalar.copy(zb[:], pzb[:])
                        nc.vector.tensor_mul(out=aoT[h * HD:(h + 1) * HD, :],
                                             in0=pv[0:HD, :], in1=zb[:])
                    # o_proj partial for this q-chunk
                    for m in range(KT):
                        po = pso.tile([P, QC], F32, tag="o")
                        nc.tensor.matmul(po[:], lhsT=wo_sb[:, m * P:(m + 1) * P],
                                         rhs=aoT[:], start=True, stop=True)
                        ob = wk.tile([P, QC], BF16, tag="ob")
                        nc.vector.tensor_copy(ob[:], po[:])
                        nc.sync.dma_start(o1c[n][m * P:(m + 1) * P, :], ob[:])
                    nc.gpsimd.collective_compute(
                        "AllReduce", mybir.AluOpType.add,
                        replica_groups=[list(range(N_CORES))],
                        ins=[o1c[n].opt()], outs=[o1sc[n].opt()],
                    )

        # ============ residual + RMS norm (redundant on all cores) ============
        def rms_norm(osrc_chunks, pool_tag, recast, out_dram=None):
            with tc.tile_pool(name=f"res_n{pool_tag}", bufs=3) as resn, \
                 tc.tile_pool(name=f"wk_n{pool_tag}", bufs=2) as wkn, \
                 tc.tile_pool(name=f"ps_ss{pool_tag}", bufs=1, space="PSUM") as pss:
                for j in range(4):
                    sl = slice(j * 512, (j + 1) * 512)
                    ss = pss.tile([1, 512], F32, tag="ss", name=f"ss{j}")
                    for t in range(KT):
                        ot = resn.tile([P, 512], BF16, tag="res")
                        nc.sync.dma_start(ot[:], osrc_chunks[j][t * P:(t + 1) * P, :])
                        nc.vector.tensor_add(out=xt[:, t, sl], in0=xt[:, t, sl],
                                             in1=ot[:])
                        sq = resn.tile([P, 512], BF16, tag="res2")
                        nc.scalar.square(sq[:], xt[:, t, sl])
                        nc.tensor.matmul(ss[:], lhsT=onesb[:], rhs=sq[:],
                                         start=(t == 0), stop=(t == KT - 1))
                    srow = wkn.tile([1, 512], F32, tag="srow")
                    nc.scalar.activation(srow[:], ss[:],
                                         mybir.ActivationFunctionType.Sqrt,
                                         bias=eps_sb, scale=1.0 / HID)
                    rrow = wkn.tile([1, 512], F32, tag="rrow")
                    nc.vector.reciprocal(rrow[:], srow[:])
                    prb = pss.tile([P, 512], F32, tag="rbb")
                    nc.tensor.matmul(prb[:], lhsT=onesr[:], rhs=rrow[:],
                                     start=True, stop=True)
                    rb = wkn.tile([P, 512], F32, tag="rb")
                    nc.scalar.copy(rb[:], prb[:])
                    for t in range(KT):
                        if recast:
                            nc.vector.tensor_mul(out=xtbc[j][:, t, :],
                                                 in0=xt[:, t, sl], in1=rb[:])
                        if t % 2 == 0:
                            nc.gpsimd.tensor_tensor(xt[:, t, sl], xt[:, t, sl],
                                                    rb[:], mybir.AluOpType.mult)
                        else:
                            nc.vector.tensor_mul(out=xt[:, t, sl],
                                                 in0=xt[:, t, sl], in1=rb[:])
                        if out_dram is not None:
                            nc.sync.dma_start(out_dram[:, t, sl], xt[:, t, sl])

        # ================= SwiGLU MLP (intermediate-sharded) =================
        # MLP pools open before norm1 so chunk-n matmuls overlap the norm/AR tail.
        NI = IP // P  # 3
        with tc.tile_pool(name="mlp", bufs=1) as mlp, \
             tc.tile_pool(name="wk_m", bufs=2) as wkm, \
             tc.tile_pool(name="ps_gu", bufs=4, space="PSUM") as psg, \
             tc.tile_pool(name="ps_d", bufs=2, space="PSUM") as psd:
            wgu_sb = mlp.tile([P, KT, 2 * IP], BF16, tag="wgu")
            wdn_sb = mlp.tile([P, NI, HID], BF16, tag="wdn")
            nc.sync.dma_start(wgu_sb[:], wgu.rearrange("(t p) m -> p t m", p=P))
            nc.sync.dma_start(wdn_sb[:], wdn.rearrange("(t p) m -> p t m", p=P))

            rms_norm(o1sc, "1", True)

            for n in range(4):
                sl = slice(n * 512, (n + 1) * 512)
                actT = wkm.tile([P, NI, 512], BF16, tag="actT")
                for g in range(NI):
                    psgt = psg.tile([P, 512], F32, tag="gu")
                    for kt in range(KT):
                        nc.tensor.matmul(psgt[:],
                                         lhsT=wgu_sb[:, kt, g * P:(g + 1) * P],
                                         rhs=xtbc[n][:, kt, :],
                                         start=(kt == 0), stop=(kt == KT - 1))
                    psut = psg.tile([P, 512], F32, tag="gu")
                    for kt in range(KT):
                        nc.tensor.matmul(psut[:],
                                         lhsT=wgu_sb[:, kt, (NI + g) * P:(NI + g + 1) * P],
                                         rhs=xtbc[n][:, kt, :],
                                         start=(kt == 0), stop=(kt == KT - 1))
                    nc.scalar.activation(actT[:, g, :], psgt[:],
                                         mybir.ActivationFunctionType.Silu)
                    nc.vector.tensor_mul(out=actT[:, g, :], in0=actT[:, g, :],
                                         in1=psut[:])
                for m in range(KT):
                    ps = psd.tile([P, 512], F32, tag="d")
                    for kt in range(NI):
                        nc.tensor.matmul(ps[:],
                                         lhsT=wdn_sb[:, kt, m * P:(m + 1) * P],
                                         rhs=actT[:, kt, :],
                                         start=(kt == 0), stop=(kt == NI - 1))
                    db = wkm.tile([P, 512], BF16, tag="db")
                    nc.scalar.copy(db[:], ps[:])
                    nc.sync.dma_start(o2c[n][m * P:(m + 1) * P, :], db[:])
                nc.gpsimd.collective_compute(
                    "AllReduce", mybir.AluOpType.add,
                    replica_groups=[list(range(N_CORES))],
                    ins=[o2c[n].opt()], outs=[o2sc[n].opt()],
                )

        rms_norm(o2sc, "2", False,
                 out_dram=outT.rearrange("(t p) s -> p t s", p=P))


def _dma_bf(nc, pool, src, shape, tag):
    """DMA an f32 DRAM tensor into a shared scratch f32 tile (caller converts)."""
    t = pool.tile([P, S], F32, tag="fscratch", name=tag)
    nc.sync.dma_start(t[:shape[0], :shape[1]], src[:])
    return t[:shape[0], :shape[1]]


def _prep_inputs(cos, sin, hidden_states, attn_bias, w_qkv, w_o, w_gate_up, w_down):
    xT = np.ascontiguousarray(hidden_states.reshape(S, HID).T).astype(np.float32)
    cosT = np.ascontiguousarray(cos.T).astype(np.float32)
    sinT = np.ascontiguousarray(sin.T).astype(np.float32)
    cs2 = np.concatenate([cosT, cosT], axis=0)
    sn2 = np.concatenate([sinT, sinT], axis=0)
    # rotate_half as a left-multiply in transposed layout: R2 = blockdiag(R, R)
    R = np.zeros((HD, HD), np.float32)
    H2 = HD // 2
    for i in range(H2):
        R[i, i + H2] = -1.0
        R[i + H2, i] = 1.0
    R2 = np.zeros((2 * HD, 2 * HD), np.float32)
    R2[:HD, :HD] = R
    R2[HD:, HD:] = R
    r2t = np.ascontiguousarray(R2.T)

    in_maps = []
    ISH = INTER // N_CORES  # 352
    for c in range(N_CORES):
        hA = HPC * c
        qcols = w_qkv[:, hA * HD:(hA + HPC) * HD]
        kcols = w_qkv[:, (NH + hA) * HD:(NH + hA + HPC) * HD]
        vcols = w_qkv[:, (2 * NH + hA) * HD:(2 * NH + hA + HPC) * HD]
        wqkv_c = np.ascontiguousarray(
            np.concatenate([qcols, kcols, vcols], axis=1), np.float32)
        wo_c = np.ascontiguousarray(w_o[hA * HD:(hA + HPC) * HD, :], np.float32)
        bT = attn_bias[0, hA:hA + HPC].transpose(0, 2, 1)  # [h][k][q]
        expb_c = np.exp(bT).astype(ml_dtypes.bfloat16)
        wg = w_gate_up[:, c * ISH:(c + 1) * ISH]
        wu = w_gate_up[:, INTER + c * ISH:INTER + (c + 1) * ISH]
        wgu_c = np.zeros((HID, 2 * IP), np.float32)
        wgu_c[:, :ISH] = wg
        wgu_c[:, IP:IP + ISH] = wu
        wdn_c = np.zeros((IP, HID), np.float32)
        wdn_c[:ISH] = w_down[c * ISH:(c + 1) * ISH, :]
        in_maps.append({
            "xT": xT, "wqkv": wqkv_c, "wo": wo_c, "cs2": cs2, "sn2": sn2,
            "r2t": r2t, "expb": np.ascontiguousarray(expb_c),
            "wgu": wgu_c.astype(ml_dtypes.bfloat16),
            "wdn": wdn_c.astype(ml_dtypes.bfloat16),
        })
    return in_maps


def kernel(cos, sin, hidden_states, attn_bias, w_qkv, w_o, w_gate_up, w_down,
           **_ignored):
    args = [np.asarray(a, np.float32) for a in
            (cos, sin, hidden_states, attn_bias, w_qkv, w_o, w_gate_up, w_down)]
    if "nc" not in _cache:
        _cache["nc"] = _build()
    nc = _cache["nc"]
    in_maps = _prep_inputs(*args)
    res = run_bass_kernel_spmd(nc, in_maps, core_ids=list(range(N_CORES)))
    _cache["last_results"] = res
    outT = res.results[0]["outT"]
    return np.ascontiguousarray(outT.T).reshape(1, S, HID).astype(np.float32)



# revision 31
# speedup vs baseline: 1.4297x; 1.4297x over previous
"""Fused transformer block (attention + SwiGLU MLP, RMS norms) on 8 TRN2 NeuronCores.

Sequence-parallel sharding: every core computes ALL 16 heads of attention and
the FULL SwiGLU MLP, but only for its own 256 query positions.  The only
collective is one fp8 AllGather of K/V (each core projects k,v for its 2
heads over all 2048 positions, RoPEs k, transposes v, and contributes both).
The core's OWN head-pair block is processed straight from the local SBUF
tiles while the AllGather is in flight; the other 7 blocks are read from the
gathered buffer with rank-relative dynamic indices ((rank+j) % 8), so the
program is identical on every core.  o_proj/MLP outputs are complete
per-core, so residuals + RMS norms are local and the full output is
assembled host-side from per-core column slices — no AllReduce anywhere.

Layouts are feature-major ([feature, seq]) so every matmul contracts along
SBUF partitions.  Softmax: p = exp(qk/8) * expb with expb = exp(bias^T)
pre-computed host-side in bf16; row sums come free via an appended
ones-column in the PV matmul.  K/V live in fp8e4m3 (their error is diluted
~14x by the residual); everything else is bf16 with f32 accumulation.
"""

import sys

sys.path.insert(0, "/opt/trn_rl_repo")

import numpy as np
import ml_dtypes

import concourse.bass as bass
import concourse.mybir as mybir
import concourse.tile as tile
from concourse import bacc
from concourse.bass_utils import run_bass_kernel_spmd
from concourse.masks import make_identity

P = 128
S = 2048
HID = 1024
NH = 16
HD = 64
INTER = 2816
EPS = 1e-5
N_CORES = 8
SQ = S // N_CORES            # own query positions per core = 256
KT = HID // P                # 8 hid k-tiles
KB = S // P                  # 16 k-blocks
NBLK = NH // 2               # 8 head-pair blocks
IB = INTER // P              # 22 intermediate blocks
HB = S * P                   # one k-or-v half-block, flat elements
F32 = mybir.dt.float32
BF16 = mybir.dt.bfloat16
F8 = mybir.dt.float8e4
Exp = mybir.ActivationFunctionType.Exp
Silu = mybir.ActivationFunctionType.Silu
Sqrt = mybir.ActivationFunctionType.Sqrt

_cache = {}
_DEBUG = False


def _build():
    nc = bacc.Bacc("TRN2", target_bir_lowering=False, debug=False,
                   num_devices=N_CORES)
    xbT = nc.dram_tensor("xbT", [HID, S], BF16, kind="ExternalInput").ap()
    xoT = nc.dram_tensor("xoT", [HID, SQ], BF16, kind="ExternalInput").ap()
    wq = nc.dram_tensor("wq", [HID, HID + 4 * HD], BF16, kind="ExternalInput").ap()
    wo = nc.dram_tensor("wo", [HID, HID], BF16, kind="ExternalInput").ap()
    cso = nc.dram_tensor("cso", [P, SQ], BF16, kind="ExternalInput").ap()
    sno = nc.dram_tensor("sno", [P, SQ], BF16, kind="ExternalInput").ap()
    csf = nc.dram_tensor("csf", [P, S], BF16, kind="ExternalInput").ap()
    snf = nc.dram_tensor("snf", [P, S], BF16, kind="ExternalInput").ap()
    r2t = nc.dram_tensor("r2t", [P, P], BF16, kind="ExternalInput").ap()
    expb = nc.dram_tensor("expb", [NH, S, SQ], BF16, kind="ExternalInput").ap()
    wgu = nc.dram_tensor("wgu", [HID, IB, 2 * P], BF16, kind="ExternalInput").ap()
    wdn = nc.dram_tensor("wdn", [INTER, HID], BF16, kind="ExternalInput").ap()
    outT = nc.dram_tensor("outT", [HID, SQ], F32, kind="ExternalOutput").ap()
    dbg = None
    if _DEBUG:
        dbg = {
            "dao": nc.dram_tensor("dao", [HID, SQ], BF16, kind="ExternalOutput").ap(),
            "dx1": nc.dram_tensor("dx1", [HID, SQ], BF16, kind="ExternalOutput").ap(),
        }

    with tile.TileContext(nc) as tc:
        _body(nc, tc, xbT, xoT, wq, wo, cso, sno, csf, snf, r2t, expb, wgu,
              wdn, outT, dbg)
    nc.compile()
    return nc


def _body(nc, tc, xbT, xoT, wq, wo, cso, sno, csf, snf, r2t, expb, wgu, wdn,
          outT, dbg=None):
    nc.cache_partition_id()
    rank = nc.partition_id()
    with tc.tile_pool(name="const", bufs=1) as const, \
         tc.tile_pool(name="dram1", bufs=1, space="DRAM") as dram1:
        misc = const.tile([P, 2], F32, tag="misc")
        onesb = const.tile([P, 1], BF16, tag="onesb")
        onesr = const.tile([1, P], F32, tag="onesr")
        idb = const.tile([P, P], BF16, tag="idb")
        qT = const.tile([P, KT, SQ], BF16, tag="qT")       # q, all heads
        xt = const.tile([P, KT, SQ], BF16, tag="xt")       # own x -> x1 -> mlp-in
        cs_o = const.tile([P, SQ], BF16, tag="cs_o")
        sn_o = const.tile([P, SQ], BF16, tag="sn_o")
        r2b = const.tile([P, P], BF16, tag="r2b")
        kTt = const.tile([P, S], F8, tag="kTt")            # own roped k, fp8
        vab0 = const.tile([P, KB, 2 * (HD + 1)], F8, tag="vab0")
        eps_sb = misc[0:1, 0:1]
        nc.gpsimd.memset(eps_sb, EPS)
        nc.gpsimd.memset(onesb[:], 1.0)
        nc.gpsimd.memset(onesr[:], 1.0)
        nc.gpsimd.memset(vab0[:, :, HD], 1.0)
        nc.gpsimd.memset(vab0[:, :, 2 * HD + 1], 1.0)
        make_identity(nc, idb[:])
        nc.sync.dma_start(xt[:], xoT.rearrange("(t p) q -> p t q", p=P))
        nc.sync.dma_start(cs_o[:], cso)
        nc.sync.dma_start(sn_o[:], sno)
        nc.sync.dma_start(r2b[:], r2t)

        kvin = dram1.tile([2 * HB], F8, tag="kvin")
        kvs = dram1.tile([2 * N_CORES * HB], F8, tag="kvs",
                         addr_space="Shared")

        # ================= qkv projection + rope + v transpose ==============
        with tc.tile_pool(name="proj", bufs=1) as proj, \
             tc.tile_pool(name="xch", bufs=2) as xch, \
             tc.tile_pool(name="wk_p", bufs=3) as wkp, \
             tc.tile_pool(name="ps_p", bufs=3, space="PSUM") as psp, \
             tc.tile_pool(name="ps_t", bufs=2, space="PSUM") as pst:
            wq_sb = proj.tile([P, KT, HID + 4 * HD], BF16, tag="wq")
            wqr = wq.rearrange("(t p) m -> p t m", p=P)
            # k/v weight columns first (they gate the kv projection)
            nc.sync.dma_start(wq_sb[:, 0:4, HID:HID + 2 * P],
                              wqr[:, 0:4, HID:HID + 2 * P])
            nc.sync.dma_start(wq_sb[:, 4:8, HID:HID + 2 * P],
                              wqr[:, 4:8, HID:HID + 2 * P])
            for j in range(4):
                nc.sync.dma_start(wq_sb[:, 2 * j:2 * j + 2, 0:HID],
                                  wqr[:, 2 * j:2 * j + 2, 0:HID])
            vT = proj.tile([P, S], BF16, tag="vT")
            cs_f = proj.tile([P, S], BF16, tag="cs_f")
            sn_f = proj.tile([P, S], BF16, tag="sn_f")
            nc.sync.dma_start(cs_f[:], csf)
            nc.sync.dma_start(sn_f[:], snf)

            # ---- k,v projection over all 2048 positions (own 2 heads) ----
            for n in range(4):
                sl = slice(n * 512, (n + 1) * 512)
                xc = xch.tile([P, KT, 512], BF16, tag="xc")
                xsr = xbT[:, sl].rearrange("(t p) s -> p t s", p=P)
                for jj in range(4):
                    nc.sync.dma_start(xc[:, 2 * jj:2 * jj + 2, :],
                                      xsr[:, 2 * jj:2 * jj + 2, :])
                psk = psp.tile([P, 512], F32, tag="pp")
                for kt in range(KT):
                    nc.tensor.matmul(psk[:], lhsT=wq_sb[:, kt, HID:HID + P],
                                     rhs=xc[:, kt, :],
                                     start=(kt == 0), stop=(kt == KT - 1))
                kpre = wkp.tile([P, 512], BF16, tag="kpre")
                nc.scalar.copy(kpre[:], psk[:])
                psv = psp.tile([P, 512], F32, tag="pp")
                for kt in range(KT):
                    nc.tensor.matmul(psv[:],
                                     lhsT=wq_sb[:, kt, HID + P:HID + 2 * P],
                                     rhs=xc[:, kt, :],
                                     start=(kt == 0), stop=(kt == KT - 1))
                nc.vector.tensor_copy(vT[:, sl], psv[:])
                # rope(k): k*cos + (R2 @ k)*sin -> fp8
                psr = psp.tile([P, 512], F32, tag="pp")
                nc.tensor.matmul(psr[:], lhsT=r2b[:], rhs=kpre[:],
                                 start=True, stop=True)
                m1 = wkp.tile([P, 512], BF16, tag="m1")
                m2 = wkp.tile([P, 512], BF16, tag="m2")
                nc.vector.tensor_mul(out=m1[:], in0=kpre[:], in1=cs_f[:, sl])
                nc.vector.tensor_mul(out=m2[:], in0=psr[:], in1=sn_f[:, sl])
                nc.vector.tensor_add(out=kTt[:, sl], in0=m1[:], in1=m2[:])
                # transpose this chunk's v into vab0 [k, (v|1)] fp8 blocks
                for t in range(4):
                    kb = 4 * n + t
                    pt = pst.tile([P, P], BF16, tag="tr")
                    nc.tensor.transpose(pt[:], vT[:, kb * P:(kb + 1) * P],
                                        idb[:])
                    if t % 2 == 0:
                        nc.vector.tensor_copy(vab0[:, kb, 0:HD], pt[:, 0:HD])
                        nc.vector.tensor_copy(vab0[:, kb, HD + 1:2 * HD + 1],
                                              pt[:, HD:2 * HD])
                    else:
                        nc.scalar.copy(vab0[:, kb, 0:HD], pt[:, 0:HD])
                        nc.scalar.copy(vab0[:, kb, HD + 1:2 * HD + 1],
                                       pt[:, HD:2 * HD])

            # ---- contribute own k/v and gather all ----
            kvw = kvin[0:HB].rearrange("(p s) -> p s", p=P)
            for j in range(4):
                sl = slice(j * 512, (j + 1) * 512)
                nc.sync.dma_start(kvw[:, sl], kTt[:, sl])
            vw = kvin[HB:2 * HB].rearrange("(kb p d) -> p kb d", p=P, d=P)
            for j in range(2):
                sl = slice(j * 8, (j + 1) * 8)
                nc.sync.dma_start(vw[:, sl, 0:HD], vab0[:, sl, 0:HD])
                nc.sync.dma_start(vw[:, sl, HD:2 * HD],
                                  vab0[:, sl, HD + 1:2 * HD + 1])
            nc.gpsimd.collective_compute(
                "AllGather", mybir.AluOpType.bypass,
                replica_groups=[list(range(N_CORES))],
                ins=[kvin.opt()], outs=[kvs.opt()],
            )

            # ---- q projection for own positions (all 16 heads), during AG --
            for m in range(KT):
                psq = psp.tile([P, SQ], F32, tag="pq")
                for kt in range(KT):
                    nc.tensor.matmul(psq[:], lhsT=wq_sb[:, kt, m * P:(m + 1) * P],
                                     rhs=xt[:, kt, :],
                                     start=(kt == 0), stop=(kt == KT - 1))
                qpre = wkp.tile([P, SQ], BF16, tag="qpre")
                nc.scalar.copy(qpre[:], psq[:])
                psr = psp.tile([P, SQ], F32, tag="pq")
                nc.tensor.matmul(psr[:], lhsT=r2b[:], rhs=qpre[:],
                                 start=True, stop=True)
                m1 = wkp.tile([P, SQ], BF16, tag="qm1")
                m2 = wkp.tile([P, SQ], BF16, tag="qm2")
                nc.vector.tensor_mul(out=m1[:], in0=qpre[:], in1=cs_o[:])
                nc.vector.tensor_mul(out=m2[:], in0=psr[:], in1=sn_o[:])
                nc.vector.tensor_add(out=qT[:, m, :], in0=m1[:], in1=m2[:])

        # ======================= attention + o_proj =========================
        with tc.tile_pool(name="wts", bufs=1) as wts:
            wo_sb = wts.tile([P, KT, HID], BF16, tag="wo")
            wdn_sb = wts.tile([P, IB, HID], BF16, tag="wdn")
            for j in range(4):
                nc.sync.dma_start(
                    wo_sb[:, 2 * j:2 * j + 2, :],
                    wo.rearrange("(t p) m -> p t m", p=P)[:, 2 * j:2 * j + 2, :])
            wdnr = wdn.rearrange("(b p) m -> p b m", p=P)
            for j in range(8):
                b0, b1 = (IB * j) // 8, (IB * (j + 1)) // 8
                nc.sync.dma_start(wdn_sb[:, b0:b1, :], wdnr[:, b0:b1, :])
            wgupre = wts.tile([P, 6, KT, 2 * P], BF16, tag="wgupre")
            wgr_all = wgu.rearrange("(t p) b m -> p b t m", p=P)
            for bb in range(6):
                nc.sync.dma_start(wgupre[:, bb, :, :], wgr_all[:, bb, :, :])
            wgup_cm = tc.tile_pool(name="wgup", bufs=4)
            wgup = wgup_cm.__enter__()

            with tc.tile_pool(name="att", bufs=1) as att, \
                 tc.tile_pool(name="kvb", bufs=3) as kvb, \
                 tc.tile_pool(name="ebp", bufs=3) as ebp, \
                 tc.tile_pool(name="ptp", bufs=2) as ptp, \
                 tc.tile_pool(name="wk_a", bufs=2) as wka:
                aoT = att.tile([P, KT, SQ], BF16, tag="aoT")
                _att_blocks(nc, tc, att, kvb, ebp, ptp, wka, aoT, kvs, expb,
                            qT, onesr, idb, rank, kTt, vab0)
                nc.multi_engine_barrier(
                    [mybir.EngineType.PE, mybir.EngineType.DVE,
                     mybir.EngineType.Activation])
                if dbg is not None:
                    nc.sync.dma_start(
                        dbg["dao"].rearrange("(t p) q -> p t q", p=P), aoT[:])

                # ---- o_proj + residual + rms norm 1 (all local) ----
                with tc.tile_pool(name="ps_o", bufs=4, space="PSUM") as pso, \
                     tc.tile_pool(name="ps_n", bufs=1, space="PSUM") as psn:
                    ss = psn.tile([1, SQ], F32, tag="ss")
                    for m in range(KT):
                        po = pso.tile([P, SQ], F32, tag="po")
                        for ft in range(KT):
                            nc.tensor.matmul(po[:],
                                             lhsT=wo_sb[:, ft, m * P:(m + 1) * P],
                                             rhs=aoT[:, ft, :],
                                             start=(ft == 0), stop=(ft == KT - 1))
                        nc.vector.tensor_add(out=xt[:, m, :], in0=po[:],
                                             in1=xt[:, m, :])
                        sq = wka.tile([P, SQ], BF16, tag="sq")
                        nc.scalar.square(sq[:], xt[:, m, :])
                        nc.tensor.matmul(ss[:], lhsT=onesb[:], rhs=sq[:],
                                         start=(m == 0), stop=(m == KT - 1))
                    srow = wka.tile([1, SQ], F32, tag="srow")
                    nc.scalar.activation(srow[:], ss[:], Sqrt, bias=eps_sb,
                                         scale=1.0 / HID)
                    rr = wka.tile([1, SQ], F32, tag="rr")
                    nc.vector.reciprocal(rr[:], srow[:])
                    prb = psn.tile([P, SQ], F32, tag="prb", name="prb")
                    nc.tensor.matmul(prb[:], lhsT=onesr[:], rhs=rr[:],
                                     start=True, stop=True)
                    rb = wka.tile([P, SQ], F32, tag="rb")
                    nc.scalar.copy(rb[:], prb[:])
                    for m in range(KT):
                        nc.vector.tensor_mul(out=xt[:, m, :], in0=xt[:, m, :],
                                             in1=rb[:])
                    if dbg is not None:
                        nc.sync.dma_start(
                            dbg["dx1"].rearrange("(t p) q -> p t q", p=P),
                            xt[:])

            # ========================= SwiGLU MLP ===========================
            with tc.tile_pool(name="mlp", bufs=1) as mlp, \
                 tc.tile_pool(name="wk_m", bufs=3) as wkm, \
                 tc.tile_pool(name="ps_g", bufs=4, space="PSUM") as psg, \
                 tc.tile_pool(name="ps_d", bufs=2, space="PSUM") as psd, \
                 tc.tile_pool(name="ps_n2", bufs=1, space="PSUM") as psn2:
                act = mlp.tile([P, IB, SQ], BF16, tag="act")
                xo = mlp.tile([P, KT, SQ], F32, tag="xo")
                for bb in range(IB):
                    if bb < 6:
                        wgusb = wgupre[:, bb, :, :]
                    else:
                        wgt = wgup.tile([P, KT, 2 * P], BF16, tag="wgusb")
                        wgr = wgu[:, bb, :].rearrange("(t p) m -> p t m", p=P)
                        nc.gpsimd.dma_start(wgt[:, 0:4, :], wgr[:, 0:4, :])
                        nc.gpsimd.dma_start(wgt[:, 4:8, :], wgr[:, 4:8, :])
                        wgusb = wgt
                    gps = psg.tile([P, 2 * SQ], F32, tag="gu")
                    for kt in range(KT):
                        nc.tensor.matmul(gps[:, 0:SQ],
                                         lhsT=wgusb[:, kt, 0:P],
                                         rhs=xt[:, kt, :],
                                         start=(kt == 0), stop=(kt == KT - 1))
                    for kt in range(KT):
                        nc.tensor.matmul(gps[:, SQ:2 * SQ],
                                         lhsT=wgusb[:, kt, P:2 * P],
                                         rhs=xt[:, kt, :],
                                         start=False, stop=(kt == KT - 1))
                    tmps = wkm.tile([P, SQ], BF16, tag="tmps")
                    nc.scalar.activation(tmps[:], gps[:, 0:SQ], Silu)
                    nc.vector.tensor_mul(out=act[:, bb, :], in0=tmps[:],
                                         in1=gps[:, SQ:2 * SQ])
                ss2 = psn2.tile([1, SQ], F32, tag="ss2")
                for m in range(KT):
                    dps = psd.tile([P, SQ], F32, tag="d")
                    for b in range(IB):
                        nc.tensor.matmul(dps[:],
                                         lhsT=wdn_sb[:, b, m * P:(m + 1) * P],
                                         rhs=act[:, b, :],
                                         start=(b == 0), stop=(b == IB - 1))
                    nc.vector.tensor_add(out=xo[:, m, :], in0=dps[:],
                                         in1=xt[:, m, :])
                    sq = wkm.tile([P, SQ], BF16, tag="sq2")
                    nc.scalar.square(sq[:], xo[:, m, :])
                    nc.tensor.matmul(ss2[:], lhsT=onesb[:], rhs=sq[:],
                                     start=(m == 0), stop=(m == KT - 1))
                srow = wkm.tile([1, SQ], F32, tag="srow2")
                nc.scalar.activation(srow[:], ss2[:], Sqrt, bias=eps_sb,
                                     scale=1.0 / HID)
                rr = wkm.tile([1, SQ], F32, tag="rr2")
                nc.vector.reciprocal(rr[:], srow[:])
                prb = psn2.tile([P, SQ], F32, tag="prb2", name="prb2")
                nc.tensor.matmul(prb[:], lhsT=onesr[:], rhs=rr[:],
                                 start=True, stop=True)
                rb = wkm.tile([P, SQ], F32, tag="rb2")
                nc.scalar.copy(rb[:], prb[:])
                outr = outT.rearrange("(t p) q -> p t q", p=P)
                for m in range(KT):
                    nc.vector.tensor_mul(out=xo[:, m, :], in0=xo[:, m, :],
                                         in1=rb[:])
                    nc.sync.dma_start(outr[:, m, :], xo[:, m, :])
            wgup_cm.__exit__(None, None, None)


def _head(nc, ebp, ptp, wka, pss, psv_, psz, ao_ap, kT_ap, vab, eb_src, q_ap,
          hh, onesr):
    """One attention head: QK^T (4 kb-quads), exp*bias, PV + row sums, 1/z."""
    hsl = slice(hh * HD, (hh + 1) * HD)
    eb = ebp.tile([P, KB, SQ], BF16, tag="eb")
    for quad in range(4):
        qsl = slice(4 * quad, 4 * quad + 4)
        if quad % 2 == 0:
            nc.scalar.dma_start(eb[:, qsl, :], eb_src[:, qsl, :])
        else:
            nc.sync.dma_start(eb[:, qsl, :], eb_src[:, qsl, :])
    pT = ptp.tile([P, KB, SQ], BF16, tag="pT")
    for quad in range(4):
        ps = pss.tile([P, 4 * SQ], F32, tag="sc")
        for j in range(4):
            kb = 4 * quad + j
            # ps spans 2 PSUM banks; each bank's first matmul must start.
            nc.tensor.matmul(
                ps[:, j * SQ:(j + 1) * SQ],
                lhsT=kT_ap[hsl, kb * P:(kb + 1) * P],
                rhs=q_ap[hsl, :],
                start=(j % 2 == 0), stop=(j == 3),
                tile_position=(hh * HD, 0),
            )
        psv4 = ps[:].rearrange("p (a q) -> p a q", a=4)
        nc.scalar.activation(pT[:, 4 * quad:4 * quad + 4, :], psv4, Exp,
                             scale=0.125)
        ebsl = eb[:, 4 * quad:4 * quad + 4, :]
        ptsl = pT[:, 4 * quad:4 * quad + 4, :]
        nc.vector.tensor_mul(out=ptsl, in0=ptsl, in1=ebsl)
    pv = psv_.tile([HD + 1, SQ], F32, tag="pv")
    a0 = hh * (HD + 1)
    for kb in range(KB):
        nc.tensor.matmul(pv[:], lhsT=vab[:, kb, a0:a0 + HD + 1],
                         rhs=pT[:, kb, :],
                         start=(kb == 0), stop=(kb == KB - 1))
    zr = wka.tile([1, SQ], F32, tag="zr")
    nc.vector.reciprocal(zr[:], pv[HD:HD + 1, :])
    pzb = psz.tile([HD, SQ], F32, tag="zb")
    nc.tensor.matmul(pzb[:], lhsT=onesr[:, :HD], rhs=zr[:],
                     start=True, stop=True)
    zb = wka.tile([HD, SQ], F32, tag="zbs")
    nc.scalar.copy(zb[:], pzb[:])
    nc.vector.tensor_mul(out=ao_ap[hsl, :], in0=pv[0:HD, :], in1=zb[:])


def _att_blocks(nc, tc, att, kvb, ebp, ptp, wka, aoT, kvs, expb, qT, onesr,
                idb, rank, kTt, vab0):
    """All 8 head-pair blocks: own block from local tiles (during the AG),
    the rest via rank-relative dynamic reads of the gathered buffer."""
    with tc.tile_pool(name="ps_s", bufs=2, space="PSUM") as pss, \
         tc.tile_pool(name="ps_v", bufs=2, space="PSUM") as psv_, \
         tc.tile_pool(name="ps_z", bufs=1, space="PSUM") as psz:
        for j in range(NBLK):
            idx = (rank + j) % NBLK
            if j == 0:
                kT_ap = kTt
                vab = vab0
            else:
                kTb = kvb.tile([P, S], F8, tag="kTb")
                vab = kvb.tile([P, KB, 2 * (HD + 1)], F8, tag="vab")
                koff = idx * (2 * HB)
                ksr = kvs[bass.ds(koff, HB)].rearrange("(p s) -> p s", p=P)
                # gpsimd queue: these execute strictly after the collective's
                # trigger+wait instruction, so they can never race the gather.
                nc.gpsimd.dma_start(kTb[:, 0:1024], ksr[:, 0:1024])
                nc.gpsimd.dma_start(kTb[:, 1024:2048], ksr[:, 1024:2048])
                vsr = kvs[bass.ds(koff + HB, HB)].rearrange(
                    "(kb p d) -> p kb d", p=P, d=P)
                nc.gpsimd.dma_start(vab[:, 0:8, 0:HD], vsr[:, 0:8, 0:HD])
                nc.gpsimd.dma_start(vab[:, 8:16, 0:HD], vsr[:, 8:16, 0:HD])
                nc.gpsimd.dma_start(vab[:, 0:8, HD + 1:2 * HD + 1],
                                    vsr[:, 0:8, HD:2 * HD])
                nc.gpsimd.dma_start(vab[:, 8:16, HD + 1:2 * HD + 1],
                                    vsr[:, 8:16, HD:2 * HD])
                nc.vector.memset(vab[:, :, HD], 1.0)
                nc.vector.memset(vab[:, :, 2 * HD + 1], 1.0)
                kT_ap = kTb
            for hh in range(2):
                eb_src = expb[2 * j + hh].rearrange("(kb p) q -> p kb q", p=P)
                _head(nc, ebp, ptp, wka, pss, psv_, psz, aoT[:, j, :],
                      kT_ap[:], vab, eb_src, qT[:, j, :], hh, onesr)
            if j == 0:
                # HAM keep-alive while the AllGather completes: a dependency
                # chain of small matmuls paced by DVE copies (~1us/link).
                ds0 = att.tile([P, P], BF16, tag="ds0")
                nc.vector.tensor_copy(ds0[:], idb[:])
                for i in range(14):
                    pdum = psz.tile([P, P], F32, tag="dum", name=f"dum{i}")
                    nc.tensor.matmul(pdum[:], lhsT=idb[:], rhs=ds0[:],
                                     start=True, stop=True)
                    nc.vector.tensor_copy(ds0[:], pdum[:])


def _prep_inputs(cos, sin, hidden_states, attn_bias, w_qkv, w_o, w_gate_up,
                 w_down):
    bf = ml_dtypes.bfloat16
    xT = np.ascontiguousarray(hidden_states.reshape(S, HID).T)
    xbT = xT.astype(bf)
    cosT = np.ascontiguousarray(cos.T)
    sinT = np.ascontiguousarray(sin.T)
    cs2 = np.concatenate([cosT, cosT], axis=0).astype(bf)
    sn2 = np.concatenate([sinT, sinT], axis=0).astype(bf)
    H2 = HD // 2
    R = np.zeros((HD, HD), np.float32)
    for i in range(H2):
        R[i, i + H2] = -1.0
        R[i + H2, i] = 1.0
    R2 = np.zeros((2 * HD, 2 * HD), np.float32)
    R2[:HD, :HD] = R
    R2[HD:, HD:] = R
    r2t = np.ascontiguousarray(R2.T).astype(bf)

    wo_b = np.ascontiguousarray(w_o).astype(bf)
    wgu_p = np.empty((HID, IB, 2 * P), np.float32)
    for b in range(IB):
        wgu_p[:, b, 0:P] = w_gate_up[:, b * P:(b + 1) * P]
        wgu_p[:, b, P:2 * P] = w_gate_up[:, INTER + b * P:INTER + (b + 1) * P]
    wgu_b = wgu_p.astype(bf)
    wdn_b = np.ascontiguousarray(w_down).astype(bf)
    ebT = np.exp(attn_bias[0].transpose(0, 2, 1)).astype(bf)  # [NH, S(k), S(q)]

    in_maps = []
    for c in range(N_CORES):
        qsl = slice(c * SQ, (c + 1) * SQ)
        # program block j holds physical head-pair (c + j) % 8 on core c
        perm = [(c + j) % NBLK for j in range(NBLK)]
        qcols = np.concatenate(
            [w_qkv[:, pj * 2 * HD:(pj + 1) * 2 * HD] for pj in perm], axis=1)
        kc = w_qkv[:, (NH + 2 * c) * HD:(NH + 2 * c + 2) * HD]
        vc = w_qkv[:, (2 * NH + 2 * c) * HD:(2 * NH + 2 * c + 2) * HD]
        wq_c = np.concatenate([qcols, kc, vc], axis=1).astype(bf)
        wo_c = np.concatenate(
            [w_o[pj * 2 * HD:(pj + 1) * 2 * HD, :] for pj in perm],
            axis=0).astype(bf)
        hperm = [2 * pj + hh for pj in perm for hh in range(2)]
        in_maps.append({
            "xbT": xbT,
            "xoT": np.ascontiguousarray(xbT[:, qsl]),
            "wq": np.ascontiguousarray(wq_c),
            "wo": np.ascontiguousarray(wo_c),
            "cso": np.ascontiguousarray(cs2[:, qsl]),
            "sno": np.ascontiguousarray(sn2[:, qsl]),
            "csf": cs2,
            "snf": sn2,
            "r2t": r2t,
            "expb": np.ascontiguousarray(ebT[hperm][:, :, qsl]),
            "wgu": wgu_b,
            "wdn": wdn_b,
        })
    return in_maps


def kernel(cos, sin, hidden_states, attn_bias, w_qkv, w_o, w_gate_up, w_down,
           **_ignored):
    args = [np.asarray(a, np.float32) for a in
            (cos, sin, hidden_states, attn_bias, w_qkv, w_o, w_gate_up, w_down)]
    if "nc" not in _cache:
        _cache["nc"] = _build()
    nc = _cache["nc"]
    in_maps = _prep_inputs(*args)
    res = run_bass_kernel_spmd(nc, in_maps, core_ids=list(range(N_CORES)))
    _cache["last_results"] = res
    out = np.empty((HID, S), np.float32)
    for c in range(N_CORES):
        out[:, c * SQ:(c + 1) * SQ] = res.results[c]["outT"]
    return np.ascontiguousarray(out.T).reshape(1, S, HID).astype(np.float32)
